# revision 1
# baseline (speedup 1.0000x reference)
"""CTRNN (6 unfolds) Trainium2 Bass kernel, data-parallel over 8 NeuronCores.

Math (per reference):
    w_x = fc_w[:, :512]; w_h = fc_w[:, 512:]
    xw  = x @ w_x^T + 0 (bias folded into tanh)
    repeat 6x:  f = tanh(xw + h @ w_h^T + b);  h = 0.9*h + 0.1*f

Device layout: everything transposed ([feature, batch]) so the recurrent
matmul needs no on-chip transposes.  Per core: batch shard of 2048.
h is kept in a rescaled representation H_t = h_t / 0.9^t so both the
xw-add and the state update are single fused scalar_tensor_tensor ops.
Matmuls run as float32r (full fp32 operands, reduced-precision multiply,
fp32 PSUM accumulate) which streams at 1 cycle/row for N=512.
"""

import numpy as np
from contextlib import ExitStack

import concourse.bass as bass
import concourse.tile as tile
import concourse.mybir as mybir
from concourse.bass_utils import run_bass_kernel_spmd


def _patch_tile_drain():
    """The walrus build in this image encodes at most one sync-wait on a
    Drain CTRL instruction; Tile's kernel-tail drain attaches one wait per
    outstanding proc and fails codegen ("Too many sync wait commands").
    Spread those waits across single-wait SP nops, then emit a bare drain."""
    if getattr(tile.TileContext, "_drain_split_patched", False):
        return
    from concourse.vector_clock import ScopedClock

    def _drain_and_barrier(self, tick_clock, wait_clock):
        nc = self.nc
        collector = nc.sync.nop(nofuse=True)
        wait_clock.add_sem_waits(
            collector.ins, ScopedClock({None: tick_clock.global_clock})
        )
        waits = list(collector.ins.sync_info.on_wait)
        del collector.ins.sync_info.on_wait[1:]
        for w in waits[1:]:
            nop = nc.sync.nop(nofuse=True)
            if nop.ins.sync_info is None:
                nop.ins.sync_info = mybir.SyncInfo(on_wait=[], on_update=[])
            nop.ins.sync_info.on_wait.append(w)
        nc.sync.drain()
        nc.all_engine_barrier()
        assert self.sems is not None
        popped = nc._tile_sem_poison_stack.pop()
        assert popped is self._sem_poison
        nc.clear_and_free_semaphores(list(self.sems.allocated().values()))
        nc.all_engine_barrier()

    tile.TileContext._drain_and_barrier = _drain_and_barrier
    tile.TileContext._drain_split_patched = True


_patch_tile_drain()


def _split_excess_waits_json(bir_json):
    """This image's walrus encodes at most ONE sync-wait per instruction
    (setupSyncWait: "Too many sync wait commands").  Tile attaches as many
    waits as deps require.  Hoist all but one wait of each instruction onto
    injected NoOps, placed just before it on the same engine."""
    import json as _json

    js = _json.loads(bir_json)
    n_split = 0
    for fn in js["functions"]:
        for blk in fn["blocks"]:
            out_insts = []
            for inst in blk["instructions"]:
                si = inst.get("sync_info") or {}
                ow = si.get("on_wait") or []
                if len(ow) > 1:
                    for w in ow[:-1]:
                        n_split += 1
                        nop = {
                            "name": f"I-ws{n_split}",
                            "opcode": "NoOp",
                            "engine": inst["engine"],
                            "ins": [],
                            "outs": [],
                            "sync_info": {"on_update": [], "on_wait": [w]},
                        }
                        if "debug" in inst:
                            nop["debug"] = inst["debug"]
                        out_insts.append(nop)
                    si["on_wait"] = [ow[-1]]
                out_insts.append(inst)
            blk["instructions"] = out_insts
    return _json.dumps(js).encode()


def _patch_compile_for_wait_cap():
    import concourse.bass_utils as _bu

    if getattr(_bu, "_wait_split_patched", False):
        return
    _orig = _bu._compile_bir_impl

    def _impl(bir_json, *args, **kwargs):
        return _orig(_split_excess_waits_json(bir_json), *args, **kwargs)

    _bu._compile_bir_impl = _impl
    _bu._wait_split_patched = True


_patch_compile_for_wait_cap()

B, D_IN, D_H = 16384, 512, 1024
N_CORES = 8
BS = B // N_CORES            # 2048 batch rows per core
UNFOLDS = 6
DT = 0.1
DECAY = 0.9                  # 1 - DT/TAU
CH = 512                     # batch chunk (matmul moving free dim)
NCH = BS // CH               # 4 chunks per core
KB = D_H // 128              # 8 hidden-dim k-blocks
KX = D_IN // 128             # 4 input-dim k-blocks
F32 = mybir.dt.float32
F32R = mybir.dt.float32r


def build_nc() -> bass.Bass:
    nc = bass.Bass()
    xT = nc.dram_tensor("xT", [D_IN, BS], F32, kind="ExternalInput")
    hT = nc.dram_tensor("hT", [D_H, BS], F32, kind="ExternalInput")
    wxT = nc.dram_tensor("wxT", [D_IN, D_H], F32, kind="ExternalInput")
    whT = nc.dram_tensor("whT", [D_H, D_H], F32, kind="ExternalInput")
    bias = nc.dram_tensor("bias", [128, KB], F32, kind="ExternalInput")
    out = nc.dram_tensor("out", [D_H, BS], F32, kind="ExternalOutput")

    with tile.TileContext(nc) as tc, ExitStack() as ctx:
        persist = ctx.enter_context(tc.tile_pool(name="persist", bufs=1))
        psum_pool = ctx.enter_context(tc.tile_pool(name="psum", bufs=8, space="PSUM"))

        # --- persistent SBUF state ---
        # h, one tile per batch chunk: [128, KB*CH]; k-block jb at cols jb*CH
        h_sb = [
            persist.tile([128, KB * CH], F32R, name=f"h_sb{c}", tag=f"h_sb{c}")
            for c in range(NCH)
        ]
        # w_h^T: [128, KB*D_H]; k-block jb at cols jb*D_H
        wh_sb = persist.tile([128, KB * D_H], F32R, name="wh_sb", tag="wh_sb")
        b_sb = persist.tile([128, KB], F32, name="b_sb", tag="b_sb")
        # xw resident in SBUF as bf16, same [feature-block, chunk] layout as h
        BF16 = mybir.dt.bfloat16
        xw_sb = [
            persist.tile([128, KB * CH], BF16, name=f"xw_sb{c}", tag=f"xw_sb{c}")
            for c in range(NCH)
        ]

        nc.sync.dma_start(out=b_sb[:], in_=bias[:, :])

        # --- phase 1: xw = x @ w_x^T, staged to DRAM (fp32 exact) ---
        # Load order matters for the head: wx + x chunk 0 gate the first
        # matmul; h/wh aren't needed until the unfold phase and load later.
        with tc.tile_pool(name="xpre", bufs=1) as xpool, \
             tc.tile_pool(name="wxpre", bufs=1) as wxpool:
            wx_sb = wxpool.tile([128, KX * D_H], F32R, name="wx_sb", tag="wx_sb")
            x_sbs = [
                xpool.tile([128, KX * CH], F32R, name="x_sb", tag=f"x_sb{c}")
                for c in range(NCH)
            ]
            # per-k-block loads, wx/x0 interleaved: with subtile deps the
            # first matmul starts once block 0 of each is resident.
            for kb in range(KX):
                nc.gpsimd.dma_start(
                    out=wx_sb[:, kb * D_H:(kb + 1) * D_H],
                    in_=wxT[kb * 128:(kb + 1) * 128, :],
                )
                nc.gpsimd.dma_start(
                    out=x_sbs[0][:, kb * CH:(kb + 1) * CH],
                    in_=xT[kb * 128:(kb + 1) * 128, 0:CH],
                )
            for c in range(1, NCH):
                nc.gpsimd.dma_start(
                    out=x_sbs[c][:].rearrange("p (kb c) -> p kb c", c=CH),
                    in_=xT[:, c * CH:(c + 1) * CH].rearrange("(kb p) c -> p kb c", p=128),
                )
            # recurrent-phase loads queue behind every precompute gate; they
            # have the whole precompute to land.
            nc.gpsimd.dma_start(
                out=wh_sb[:].rearrange("p (jb h) -> p jb h", h=D_H),
                in_=whT[:, :].rearrange("(jb p) h -> p jb h", p=128),
            )
            for hc in range(NCH):
                nc.gpsimd.dma_start(
                    out=h_sb[hc][:].rearrange("p (jb c) -> p jb c", c=CH),
                    in_=hT[:, hc * CH:(hc + 1) * CH].rearrange("(jb p) c -> p jb c", p=128),
                )
            for c in range(NCH):
                x_sb = x_sbs[c]
                for p in range(KB):
                    ps = psum_pool.tile([128, CH], F32, name="ps", tag="ps")
                    for kb in range(KX):
                        nc.tensor.matmul(
                            ps[:],
                            wx_sb[:, kb * D_H + p * 128: kb * D_H + (p + 1) * 128],
                            x_sb[:, kb * CH:(kb + 1) * CH],
                            start=(kb == 0),
                            stop=(kb == KX - 1),
                        )
                    nc.vector.tensor_copy(
                        xw_sb[c][:, p * CH:(p + 1) * CH], ps[:]
                    )

        # --- phase 2: unfold loop ---
        # opened after xpre/wxpre release so the allocator reuses their space
        fpool = ctx.enter_context(tc.tile_pool(name="fpool", bufs=3))
        stage_pool = ctx.enter_context(tc.tile_pool(name="stage", bufs=6))
        sigma = 1.0  # SBUF holds H_t = h_t / sigma
        for t in range(UNFOLDS):
            last = t == UNFOLDS - 1
            upd = DT / (sigma * DECAY)  # coefficient on f for the H update
            for c in range(NCH):
                if not last:
                    # f for the whole chunk, written per H-tile; the h update
                    # must only run after every matmul group has read old h
                    # (Jacobi, not Gauss-Seidel).
                    f_ch = fpool.tile([128, KB * CH], F32, name="f_ch", tag="f_ch", bufs=2)
                for p in range(KB):
                    if last:
                        # h6 = 0.9*sigma*H + 0.1*f, per tile, stored as soon
                        # as ready; the 0.9*sigma*H part has no dependence on
                        # this step's matmuls and runs early.
                        hs = stage_pool.tile([128, CH], F32, name="hs", tag="st")
                        nc.vector.tensor_scalar_mul(
                            hs[:], h_sb[c][:, p * CH:(p + 1) * CH],
                            float(DECAY * sigma),
                        )
                    ps = psum_pool.tile([128, CH], F32, name="ps", tag="ps")
                    for jb in range(KB):
                        nc.tensor.matmul(
                            ps[:],
                            wh_sb[:, jb * D_H + p * 128: jb * D_H + (p + 1) * 128],
                            h_sb[c][:, jb * CH:(jb + 1) * CH],
                            start=(jb == 0),
                            stop=(jb == KB - 1),
                        )
                    # z = sigma * (W @ H) + xw  (in place on psum)
                    nc.vector.scalar_tensor_tensor(
                        ps[:], ps[:], float(sigma),
                        xw_sb[c][:, p * CH:(p + 1) * CH],
                        op0=mybir.AluOpType.mult, op1=mybir.AluOpType.add,
                    )
                    if last:
                        f_t = fpool.tile([128, CH], F32, name="f_t", tag="f_t")
                        nc.scalar.activation(
                            f_t[:], ps[:], mybir.ActivationFunctionType.Tanh,
                            bias=b_sb[:, p:p + 1], scale=1.0,
                        )
                        nc.vector.scalar_tensor_tensor(
                            hs[:], f_t[:], float(DT), hs[:],
                            op0=mybir.AluOpType.mult, op1=mybir.AluOpType.add,
                        )
                        nc.sync.dma_start(
                            out=out[p * 128:(p + 1) * 128, c * CH:(c + 1) * CH],
                            in_=hs[:],
                        )
                    else:
                        nc.scalar.activation(
                            f_ch[:, p * CH:(p + 1) * CH], ps[:],
                            mybir.ActivationFunctionType.Tanh,
                            bias=b_sb[:, p:p + 1], scale=1.0,
                        )
                if not last:
                    # H += upd * f, whole chunk in one op (in place on h)
                    nc.vector.scalar_tensor_tensor(
                        h_sb[c][:], f_ch[:], float(upd), h_sb[c][:],
                        op0=mybir.AluOpType.mult, op1=mybir.AluOpType.add,
                    )
            sigma *= DECAY
    return nc


_NC_CACHE = {}


def _get_nc() -> bass.Bass:
    if "nc" not in _NC_CACHE:
        _NC_CACHE["nc"] = build_nc()
    return _NC_CACHE["nc"]


def make_in_maps(x, h, fc_w, fc_b):
    x = np.asarray(x, dtype=np.float32)
    h = np.asarray(h, dtype=np.float32)
    fc_w = np.asarray(fc_w, dtype=np.float32)
    fc_b = np.asarray(fc_b, dtype=np.float32)
    xT = np.ascontiguousarray(x.T)                    # [D_IN, B]
    hT = np.ascontiguousarray(h.T)                    # [D_H, B]
    wxT = np.ascontiguousarray(fc_w[:, :D_IN].T)      # [D_IN, D_H]
    whT = np.ascontiguousarray(fc_w[:, D_IN:].T)      # [D_H, D_H]
    bias = np.ascontiguousarray(fc_b.reshape(KB, 128).T)  # [128, KB]
    in_maps = []
    for i in range(N_CORES):
        sl = slice(i * BS, (i + 1) * BS)
        in_maps.append({
            "xT": np.ascontiguousarray(xT[:, sl]),
            "hT": np.ascontiguousarray(hT[:, sl]),
            "wxT": wxT,
            "whT": whT,
            "bias": bias,
        })
    return in_maps


def gather_out(results):
    outT = np.concatenate([results[i]["out"] for i in range(N_CORES)], axis=1)
    return np.ascontiguousarray(outT.T)  # [B, D_H]


def kernel(x, h, fc_w, fc_b):
    nc = _get_nc()
    in_maps = make_in_maps(x, h, fc_w, fc_b)
    res = run_bass_kernel_spmd(nc, in_maps, list(range(N_CORES)))
    out = gather_out(res.results)
    return (out, out)


if __name__ == "__main__":
    rng = np.random.default_rng(0)
    x = rng.standard_normal((B, D_IN), dtype=np.float32)
    h = rng.standard_normal((B, D_H), dtype=np.float32)
    fc_w = rng.standard_normal((D_H, D_IN + D_H), dtype=np.float32) / np.sqrt(D_IN + D_H)
    fc_b = np.zeros((D_H,), dtype=np.float32)
    o, _ = kernel(x, h, fc_w, fc_b)
    print(o.shape, o.dtype)



# revision 7
# speedup vs baseline: 1.1965x; 1.1965x over previous
"""CTRNN (6 unfolds) Trainium2 Bass kernel, data-parallel over 8 NeuronCores.

Math (per reference):
    w_x = fc_w[:, :512]; w_h = fc_w[:, 512:]
    xwb = x @ w_x^T + b
    repeat 6x:  f = tanh(xwb + h @ w_h^T);  h = 0.9*h + 0.1*f

Device algorithm (per core, batch shard of 2048, everything transposed to
[feature, batch]):
  Let a_t = xwb + w_h h_t (pre-activation), s_t = 10*a_t kept in bf16 SBUF.
  f_t = tanh(0.1*s_t) is written directly as fp8e4 by the scalar engine.
  s_{t+1} = 0.9*s_t + (w_h f_t + xwb)   [single-scalar DVE op: psum holds
    w_h f_t + xwb, where the "+ xwb" rides in the matmul as an fp8
    identity-block over a hi+lo split of xwb]
  h accumulates as H_t = h_t/0.9^t in bf16: H += (0.1/0.9^{t+1}) * f_t,
    split as an ACT scale-copy (g = c*f8) plus bf16 tensor_tensor adds
    spread over Pool and DVE.
  Recurrent matmuls run as fp8e4 DoubleRow (K=256/matmul, 2x rate); the
  initial a_0 = x@w_x + h0@w_h is computed once in bf16 (exact enough).
"""

import numpy as np
import ml_dtypes
from contextlib import ExitStack

import concourse.bass as bass
import concourse.tile as tile
import concourse.mybir as mybir
from concourse.bass_utils import run_bass_kernel_spmd


def _patch_tile_drain():
    """The walrus build in this image encodes at most one sync-wait on a
    Drain CTRL instruction; Tile's kernel-tail drain attaches one wait per
    outstanding proc and fails codegen ("Too many sync wait commands").
    Spread those waits across single-wait SP nops, then emit a bare drain."""
    if getattr(tile.TileContext, "_drain_split_patched", False):
        return
    from concourse.vector_clock import ScopedClock

    def _drain_and_barrier(self, tick_clock, wait_clock):
        nc = self.nc
        collector = nc.sync.nop(nofuse=True)
        wait_clock.add_sem_waits(
            collector.ins, ScopedClock({None: tick_clock.global_clock})
        )
        waits = list(collector.ins.sync_info.on_wait)
        del collector.ins.sync_info.on_wait[1:]
        for w in waits[1:]:
            nop = nc.sync.nop(nofuse=True)
            if nop.ins.sync_info is None:
                nop.ins.sync_info = mybir.SyncInfo(on_wait=[], on_update=[])
            nop.ins.sync_info.on_wait.append(w)
        nc.sync.drain()
        nc.all_engine_barrier()
        assert self.sems is not None
        popped = nc._tile_sem_poison_stack.pop()
        assert popped is self._sem_poison
        nc.clear_and_free_semaphores(list(self.sems.allocated().values()))
        nc.all_engine_barrier()

    tile.TileContext._drain_and_barrier = _drain_and_barrier
    tile.TileContext._drain_split_patched = True


_patch_tile_drain()


def _split_excess_waits_json(bir_json):
    """This image's walrus encodes at most ONE sync-wait per instruction
    (setupSyncWait: "Too many sync wait commands").  Tile attaches as many
    waits as deps require.  Hoist all but one wait of each instruction onto
    injected NoOps, placed just before it on the same engine."""
    import json as _json

    js = _json.loads(bir_json)
    n_split = 0
    for fn in js["functions"]:
        for blk in fn["blocks"]:
            out_insts = []
            for inst in blk["instructions"]:
                si = inst.get("sync_info") or {}
                ow = si.get("on_wait") or []
                if len(ow) > 1:
                    for w in ow[:-1]:
                        n_split += 1
                        nop = {
                            "name": f"I-ws{n_split}",
                            "opcode": "NoOp",
                            "engine": inst["engine"],
                            "ins": [],
                            "outs": [],
                            "sync_info": {"on_update": [], "on_wait": [w]},
                        }
                        if "debug" in inst:
                            nop["debug"] = inst["debug"]
                        out_insts.append(nop)
                    si["on_wait"] = [ow[-1]]
                out_insts.append(inst)
            blk["instructions"] = out_insts
    return _json.dumps(js).encode()


def _patch_compile_for_wait_cap():
    import concourse.bass_utils as _bu

    if getattr(_bu, "_wait_split_patched", False):
        return
    _orig = _bu._compile_bir_impl

    def _impl(bir_json, *args, **kwargs):
        return _orig(_split_excess_waits_json(bir_json), *args, **kwargs)

    _bu._compile_bir_impl = _impl
    _bu._wait_split_patched = True


_patch_compile_for_wait_cap()

B, D_IN, D_H = 16384, 512, 1024
N_CORES = 8
BS = B // N_CORES            # 2048 batch rows per core
UNFOLDS = 6
DT = 0.1
DECAY = 0.9                  # 1 - DT/TAU
CH = 512                     # batch chunk (matmul moving free dim)
NCH = BS // CH               # 4 chunks per core
KB = D_H // 128              # 8 hidden-dim feature blocks
KQ = D_H // 256              # 4 DoubleRow k-pair blocks
KX = D_IN // 128             # 4 input-dim k-blocks
F32 = mybir.dt.float32
BF16 = mybir.dt.bfloat16
FP8 = mybir.dt.float8e4
NPBF = ml_dtypes.bfloat16
NPF8 = ml_dtypes.float8_e4m3fn
DR = mybir.MatmulPerfMode.DoubleRow
Tanh = mybir.ActivationFunctionType.Tanh
Ident = mybir.ActivationFunctionType.Identity
ACopy = mybir.ActivationFunctionType.Copy
MUL = mybir.AluOpType.mult
ADD = mybir.AluOpType.add
SUB = mybir.AluOpType.subtract


def build_nc() -> bass.Bass:
    nc = bass.Bass()
    xT = nc.dram_tensor("xT", [D_IN, BS], BF16, kind="ExternalInput")
    hT = nc.dram_tensor("hT", [D_H, BS], BF16, kind="ExternalInput")
    wxT = nc.dram_tensor("wxT", [D_IN, D_H], BF16, kind="ExternalInput")
    whT = nc.dram_tensor("whT", [D_H, D_H], BF16, kind="ExternalInput")
    # fp8 w_h^T pre-packed on host into the DoubleRow stationary layout:
    # [ki, (q, ko, p, m)]
    wh8 = nc.dram_tensor("wh8", [128, KQ * KB * 2 * 128], FP8, kind="ExternalInput")
    # fp8 identity for the xwb fold: [ki, (ko, m)], both ko planes = I
    id8 = nc.dram_tensor("id8", [128, 2 * 128], FP8, kind="ExternalInput")
    bias = nc.dram_tensor("bias", [128, KB], F32, kind="ExternalInput")
    out = nc.dram_tensor("out", [D_H, BS], F32, kind="ExternalOutput")

    with tile.TileContext(nc) as tc, ExitStack() as ctx:
        persist = ctx.enter_context(tc.tile_pool(name="persist", bufs=1))
        psum_pool = ctx.enter_context(tc.tile_pool(name="psum", bufs=8, space="PSUM"))

        # --- persistent SBUF state ---
        # s = 10*(pre-activation), one tile per batch chunk: [128, KB*CH]
        s_sb = [
            persist.tile([128, KB * CH], BF16, name=f"s_sb{c}", tag=f"s_sb{c}")
            for c in range(NCH)
        ]
        # H = h/0.9^t, bf16, initialized with h0 by DMA
        H_sb = [
            persist.tile([128, KB * CH], BF16, name=f"H_sb{c}", tag=f"H_sb{c}")
            for c in range(NCH)
        ]
        # xwb hi/lo fp8, interleaved per feature block: [128, (p, ko, n)]
        xwb8 = [
            persist.tile([128, KB * 2 * CH], FP8, name=f"xwb8_{c}", tag=f"xwb8_{c}")
            for c in range(NCH)
        ]
        wh8_sb = persist.tile([128, KQ * KB * 2 * 128], FP8, name="wh8_sb", tag="wh8_sb")
        id8_sb = persist.tile([128, 2 * 128], FP8, name="id8_sb", tag="id8_sb")
        b_sb = persist.tile([128, KB], F32, name="b_sb", tag="b_sb")

        nc.sync.dma_start(out=b_sb[:], in_=bias[:, :])
        nc.sync.dma_start(out=id8_sb[:], in_=id8[:, :])

        # --- phase A: bf16 precompute of xwb (fp8 hi+lo) and s0 ---
        with tc.tile_pool(name="xpre", bufs=1) as xpool, \
             tc.tile_pool(name="wpre", bufs=1) as wpool, \
             tc.tile_pool(name="xwbpre", bufs=2) as xwbpool:
            wx_sb = wpool.tile([128, KX * D_H], BF16, name="wx_sb", tag="wx_sb")
            wh_sb = wpool.tile([128, KB * D_H], BF16, name="wh_sb", tag="wh_sb")
            x_sbs = [
                xpool.tile([128, KX * CH], BF16, name="x_sb", tag=f"x_sb{c}")
                for c in range(NCH)
            ]
            # wx + x chunk0 first: they gate the first matmul
            for kb in range(KX):
                nc.gpsimd.dma_start(
                    out=wx_sb[:, kb * D_H:(kb + 1) * D_H],
                    in_=wxT[kb * 128:(kb + 1) * 128, :],
                )
                nc.gpsimd.dma_start(
                    out=x_sbs[0][:, kb * CH:(kb + 1) * CH],
                    in_=xT[kb * 128:(kb + 1) * 128, 0:CH],
                )
            for c in range(1, NCH):
                nc.gpsimd.dma_start(
                    out=x_sbs[c][:].rearrange("p (kb c) -> p kb c", c=CH),
                    in_=xT[:, c * CH:(c + 1) * CH].rearrange("(kb p) c -> p kb c", p=128),
                )
            nc.gpsimd.dma_start(
                out=wh_sb[:].rearrange("p (jb h) -> p jb h", h=D_H),
                in_=whT[:, :].rearrange("(jb p) h -> p jb h", p=128),
            )
            nc.gpsimd.dma_start(out=wh8_sb[:], in_=wh8[:, :])
            # h0 -> H tiles
            for c in range(NCH):
                nc.gpsimd.dma_start(
                    out=H_sb[c][:].rearrange("p (jb c) -> p jb c", c=CH),
                    in_=hT[:, c * CH:(c + 1) * CH].rearrange("(jb p) c -> p jb c", p=128),
                )

            # A1: xwb1 = x@wx^T + b, then fp8 hi/lo split
            for c in range(NCH):
                for p in range(KB):
                    ps = psum_pool.tile([128, CH], F32, name="ps", tag="ps")
                    for kb in range(KX):
                        nc.tensor.matmul(
                            ps[:],
                            wx_sb[:, kb * D_H + p * 128: kb * D_H + (p + 1) * 128],
                            x_sbs[c][:, kb * CH:(kb + 1) * CH],
                            start=(kb == 0),
                            stop=(kb == KX - 1),
                        )
                    xwb1 = xwbpool.tile([128, CH], BF16, name="xwb1", tag="xwb1")
                    nc.scalar.activation(xwb1[:], ps[:], Ident,
                                         bias=b_sb[:, p:p + 1], scale=1.0)
                    hi = xwb8[c][:, p * 2 * CH: p * 2 * CH + CH]
                    lo = xwb8[c][:, p * 2 * CH + CH: (p + 1) * 2 * CH]
                    nc.vector.tensor_copy(hi, xwb1[:])
                    nc.gpsimd.tensor_tensor(lo, xwb1[:], hi, op=SUB)

            # A2: s0 = 10*(h0@wh^T + xwb1)
            for c in range(NCH):
                for p in range(KB):
                    ps = psum_pool.tile([128, CH], F32, name="ps", tag="ps")
                    for jb in range(KB):
                        nc.tensor.matmul(
                            ps[:],
                            wh_sb[:, jb * D_H + p * 128: jb * D_H + (p + 1) * 128],
                            H_sb[c][:, jb * CH:(jb + 1) * CH],
                            start=(jb == 0),
                            stop=False,
                        )
                    nc.tensor.matmul(
                        ps[:],
                        id8_sb[:].rearrange("p (ko m) -> p ko m", ko=2),
                        xwb8[c][:, p * 2 * CH:(p + 1) * 2 * CH]
                            .rearrange("p (ko n) -> p ko n", ko=2),
                        start=False, stop=True, perf_mode=DR,
                    )
                    nc.scalar.activation(
                        s_sb[c][:, p * CH:(p + 1) * CH], ps[:], Ident,
                        bias=0.0, scale=10.0,
                    )

        # --- unfold loop ---
        fpool = ctx.enter_context(tc.tile_pool(name="fpool", bufs=2))
        gpool = ctx.enter_context(tc.tile_pool(name="gpool", bufs=3))
        stage = ctx.enter_context(tc.tile_pool(name="stage", bufs=6))
        sigma = 1.0  # h_t = sigma * H_t
        f8_prev = None
        for t in range(UNFOLDS):
            last = t == UNFOLDS - 1
            c2 = DT / (sigma * DECAY)  # H += c2 * f
            # f8 tiles for this step (written by ACT, read by next matmuls)
            f8_cur = [
                fpool.tile([128, KB * CH], FP8, name=f"f8_{c}", tag=f"f8_{c}")
                for c in range(NCH)
            ]
            for c in range(NCH):
                for p in range(KB):
                    if t > 0:
                        # psum = w_h f_{t-1} + xwb1  (DR fp8 + identity fold)
                        ps = psum_pool.tile([128, CH], F32, name="ps", tag="ps")
                        for q in range(KQ):
                            off = (q * KB + p) * 256
                            nc.tensor.matmul(
                                ps[:],
                                wh8_sb[:, off:off + 256]
                                    .rearrange("x (ko m) -> x ko m", ko=2),
                                f8_prev[c][:, q * 2 * CH:(q + 1) * 2 * CH]
                                    .rearrange("x (ko n) -> x ko n", ko=2),
                                start=(q == 0), stop=False, perf_mode=DR,
                            )
                        nc.tensor.matmul(
                            ps[:],
                            id8_sb[:].rearrange("x (ko m) -> x ko m", ko=2),
                            xwb8[c][:, p * 2 * CH:(p + 1) * 2 * CH]
                                .rearrange("x (ko n) -> x ko n", ko=2),
                            start=False, stop=True, perf_mode=DR,
                        )
                        # s = 0.9*s + psum
                        s_t = s_sb[c][:, p * CH:(p + 1) * CH]
                        nc.vector.scalar_tensor_tensor(
                            s_t, s_t, DECAY, ps[:], op0=MUL, op1=ADD,
                        )
                # f8 = tanh(0.1 * s), whole chunk in one ACT op
                nc.scalar.activation(
                    f8_cur[c][:], s_sb[c][:], Tanh, bias=0.0, scale=0.1,
                )
                # --- H update for chunk c ---
                if not last:
                    g = gpool.tile([128, KB * CH], BF16, name="g", tag="g")
                    if c < 3:
                        nc.scalar.activation(g[:], f8_cur[c][:], ACopy,
                                             bias=0.0, scale=float(c2))
                        nc.gpsimd.tensor_tensor(H_sb[c][:], g[:], H_sb[c][:], op=ADD)
                    else:
                        nc.vector.tensor_scalar_mul(g[:], f8_cur[c][:], float(c2))
                        nc.vector.tensor_tensor(H_sb[c][:], g[:], H_sb[c][:], op=ADD)
                else:
                    # out = 0.9^6 * (H + c2 * f)  per tile, fp32, stored
                    fs = DECAY ** UNFOLDS
                    for p in range(KB):
                        hs = stage.tile([128, CH], F32, name="hs", tag="st")
                        nc.scalar.activation(
                            hs[:], H_sb[c][:, p * CH:(p + 1) * CH], ACopy,
                            bias=0.0, scale=float(fs),
                        )
                        nc.vector.scalar_tensor_tensor(
                            hs[:], f8_cur[c][:, p * CH:(p + 1) * CH],
                            float(fs * c2), hs[:], op0=MUL, op1=ADD,
                        )
                        nc.sync.dma_start(
                            out=out[p * 128:(p + 1) * 128, c * CH:(c + 1) * CH],
                            in_=hs[:],
                        )
            f8_prev = f8_cur
            sigma *= DECAY
    return nc


_NC_CACHE = {}


def _get_nc() -> bass.Bass:
    if "nc" not in _NC_CACHE:
        _NC_CACHE["nc"] = build_nc()
    return _NC_CACHE["nc"]


def make_in_maps(x, h, fc_w, fc_b):
    x = np.asarray(x, dtype=np.float32)
    h = np.asarray(h, dtype=np.float32)
    fc_w = np.asarray(fc_w, dtype=np.float32)
    fc_b = np.asarray(fc_b, dtype=np.float32)
    xT = np.ascontiguousarray(x.T).astype(NPBF)            # [D_IN, B]
    hT = np.ascontiguousarray(h.T).astype(NPBF)            # [D_H, B]
    wxT = np.ascontiguousarray(fc_w[:, :D_IN].T).astype(NPBF)   # [D_IN, D_H]
    whT_f32 = np.ascontiguousarray(fc_w[:, D_IN:].T)       # [D_H, D_H]
    whT = whT_f32.astype(NPBF)
    # fp8 stationary pack: wh8[ki, q, p, ko, m] = whT8[q*256+ko*128+ki, p*128+m]
    whT8 = whT_f32.astype(NPF8)
    t = whT8.reshape(KQ, 2, 128, KB, 128)                  # [q, ko, ki, p, m]
    wh8 = np.ascontiguousarray(t.transpose(2, 0, 3, 1, 4).reshape(128, -1))
    id8 = np.zeros((128, 2, 128), dtype=NPF8)
    for ki in range(128):
        id8[ki, 0, ki] = 1.0
        id8[ki, 1, ki] = 1.0
    id8 = np.ascontiguousarray(id8.reshape(128, -1))
    bias = np.ascontiguousarray(fc_b.reshape(KB, 128).T)   # [128, KB]
    in_maps = []
    for i in range(N_CORES):
        sl = slice(i * BS, (i + 1) * BS)
        in_maps.append({
            "xT": np.ascontiguousarray(xT[:, sl]),
            "hT": np.ascontiguousarray(hT[:, sl]),
            "wxT": wxT,
            "whT": whT,
            "wh8": wh8,
            "id8": id8,
            "bias": bias,
        })
    return in_maps


def gather_out(results):
    outT = np.concatenate([results[i]["out"] for i in range(N_CORES)], axis=1)
    return np.ascontiguousarray(outT.T)  # [B, D_H]


def kernel(x, h, fc_w, fc_b):
    nc = _get_nc()
    in_maps = make_in_maps(x, h, fc_w, fc_b)
    res = run_bass_kernel_spmd(nc, in_maps, list(range(N_CORES)))
    out = gather_out(res.results)
    return (out, out)


if __name__ == "__main__":
    rng = np.random.default_rng(0)
    x = rng.standard_normal((B, D_IN), dtype=np.float32)
    h = rng.standard_normal((B, D_H), dtype=np.float32)
    fc_w = rng.standard_normal((D_H, D_IN + D_H), dtype=np.float32) / np.sqrt(D_IN + D_H)
    fc_b = np.zeros((D_H,), dtype=np.float32)
    o, _ = kernel(x, h, fc_w, fc_b)
    print(o.shape, o.dtype)
